# revision 2
# baseline (speedup 1.0000x reference)
"""GATv2 2-layer GNN on 8 TRN2 NeuronCores.

Strategy: dst-sorted edge sharding. Nodes are split into 8 equal contiguous
ranges (6250/core); each core owns all edges whose dst falls in its range, so
segment softmax is fully core-local. Edges are grouped into windows of 112
consecutive local dst nodes; per 128-edge tile a combined stationary
[ea^T(16); onehotT(112)] computes ea@We + xr[dst] in one matmul, an identity
matmul adds gathered xl[src], and a one-hot matmul segment-sums the
exp-weighted messages into PSUM. Self-loops are handled analytically per
window. One AllGather shares layer-2 node features across cores.
"""

import math
import numpy as np
import ml_dtypes

P = 128            # edges per tile / partitions
WN = 112           # dst nodes per window
NEG_ATT = 0.2
NEG_ACT = 0.01
NCORES = 8

bf16 = ml_dtypes.bfloat16


def _prep(x, edge_index, edge_attr):
    N, F = x.shape
    EDIM = edge_attr.shape[1]
    NL = N // NCORES                      # local nodes per core (6250)
    assert NL * NCORES == N
    NWIN = math.ceil(NL / WN)             # windows per core
    NLP = NWIN * WN                       # padded local nodes (6272)
    NTAB = NCORES * NLP
    HALF = NTAB // 2

    src_n = edge_index[0].astype(np.int64)
    dst = edge_index[1].astype(np.int64)
    deg = np.bincount(dst, minlength=N).astype(np.float64)
    invdeg_full = (1.0 / np.maximum(deg, 1.0)).astype(np.float32)

    # AG row mapping: node n -> row in the concatenated padded shard table
    agrow = (src_n // NL) * NLP + (src_n % NL)

    order = np.argsort(dst, kind="stable")
    dst_s, ea_s, agrow_s = dst[order], edge_attr[order], agrow[order]

    core_lo = np.searchsorted(dst_s, np.arange(NCORES) * NL)

    win_of_edge = (dst_s % NL) // WN
    per_core = []
    TA = TB = 1
    for c in range(NCORES):
        lo = core_lo[c]
        hi = core_lo[c + 1] if c + 1 < NCORES else len(dst_s)
        cnt = np.bincount(win_of_edge[lo:hi], minlength=NWIN)
        starts = lo + np.concatenate([[0], np.cumsum(cnt)])[:-1]
        wins = []
        for w in range(NWIN):
            s, k = int(starts[w]), int(cnt[w])
            ag = agrow_s[s:s + k]
            inA = ag < HALF
            wins.append((np.where(inA)[0] + s, np.where(~inA)[0] + s))
            TA = max(TA, math.ceil(int(inA.sum()) / P))
            TB = max(TB, math.ceil(int((~inA).sum()) / P))
        per_core.append(wins)

    T = TA + TB
    EW = T * P
    NIA, NIB = TA * P, TB * P
    maps = []
    for c in range(NCORES):
        wins = per_core[c]
        gidx = np.zeros((NWIN, EW), dtype=np.int64)
        seg = np.full((NWIN, EW), 120.0, dtype=np.float32)
        eaw = np.zeros((NWIN, EW, EDIM), dtype=np.float32)
        for w in range(NWIN):
            ia, ib = wins[w]
            for idxs, off in ((ia, 0), (ib, NIA)):
                k = len(idxs)
                gidx[w, off:off + k] = agrow_s[idxs] - (HALF if off else 0)
                seg[w, off:off + k] = (dst_s[idxs] % NL) - w * WN
                eaw[w, off:off + k] = ea_s[idxs]
        # wrapped int16 index layout for dma_gather: [16, NI//16] tiled to 128
        def wrap(a):  # a: [NWIN, NI]
            ni = a.shape[1]
            wv = a.reshape(NWIN, ni // 16, 16).transpose(0, 2, 1)  # [NWIN,16,S]
            return np.tile(wv, (1, 8, 1)).astype(np.int16)         # [NWIN,128,S]
        gidxA = wrap(gidx[:, :NIA])
        gidxB = wrap(gidx[:, NIA:])
        stat = np.zeros((NWIN, T, P, P), dtype=np.float32)
        ea_t = eaw.reshape(NWIN, T, P, EDIM)
        stat[:, :, :EDIM, :] = ea_t.transpose(0, 1, 3, 2)
        seg_t = seg.reshape(NWIN, T, P).astype(np.int32)
        w_i, t_i, p_i = np.meshgrid(np.arange(NWIN), np.arange(T), np.arange(P),
                                    indexing="ij")
        valid = seg_t < WN
        stat[w_i[valid], t_i[valid], EDIM + seg_t[valid], p_i[valid]] = 1.0

        invdeg = np.ones((NLP, 1), dtype=np.float32)
        invdeg[:NL, 0] = invdeg_full[c * NL:(c + 1) * NL]

        maps.append({
            "gidxA": gidxA, "gidxB": gidxB,
            "seg": seg.reshape(NWIN, T, P, 1),
            "stat": stat.astype(bf16),
            "ea": ea_t.astype(bf16),
            "invdeg": invdeg,
            "xT": np.pad(x[c * NL:(c + 1) * NL].T,
                         ((0, 0), (0, NLP - NL))).astype(np.float32).copy(),
        })

    meta = dict(N=N, F=F, EDIM=EDIM, NL=NL, NWIN=NWIN, NLP=NLP, T=T,
                TA=TA, TB=TB)
    return maps, meta


def _build(meta, att2):
    import concourse.bass as bass
    import concourse.bacc as bacc
    import concourse.tile as tile
    import concourse.mybir as mybir
    from concourse.masks import make_identity

    F = meta["F"]; EDIM = meta["EDIM"]; NWIN = meta["NWIN"]
    NLP = meta["NLP"]; T = meta["T"]; TA = meta["TA"]; TB = meta["TB"]
    H, C = 4, 32
    HC = H * C
    NTAB = NCORES * NLP
    FP32 = mybir.dt.float32
    BF16 = mybir.dt.bfloat16
    I32 = mybir.dt.int32
    AX = mybir.AxisListType
    ALU = mybir.AluOpType
    ACTF = mybir.ActivationFunctionType
    PAY = HC + H + EDIM                   # payload cols: weighted | ex | ea

    att2 = att2.reshape(-1)
    h_ord = np.argsort(att2 <= 0, kind="stable")
    att2_o = att2[h_ord]
    npos2 = int((att2_o > 0).sum())
    s2 = np.where(att2_o > 0, att2_o, NEG_ATT * np.abs(att2_o)).astype(np.float32)
    sgn2 = np.where(att2_o > 0, 1.0, -1.0).astype(np.float32)

    nc = bacc.Bacc("TRN2", target_bir_lowering=False, debug=False,
                   num_devices=NCORES)

    xT_e = nc.declare_dram_parameter("xT", [F, NLP], FP32, isOutput=False)
    I16 = mybir.dt.int16
    gidxA_e = nc.declare_dram_parameter("gidxA", [NWIN, P, TA * 8], I16, isOutput=False)
    gidxB_e = nc.declare_dram_parameter("gidxB", [NWIN, P, TB * 8], I16, isOutput=False)
    HALF = NCORES * NLP // 2
    seg_e = nc.declare_dram_parameter("seg", [NWIN, T, P, 1], FP32, isOutput=False)
    stat_e = nc.declare_dram_parameter("stat", [NWIN, T, P, P], BF16, isOutput=False)
    ea_e = nc.declare_dram_parameter("ea", [NWIN, T, P, EDIM], BF16, isOutput=False)
    ivd_e = nc.declare_dram_parameter("invdeg", [NLP, 1], FP32, isOutput=False)
    Wl1_e = nc.declare_dram_parameter("Wl1", [F, HC], FP32, isOutput=False)
    Wr1_e = nc.declare_dram_parameter("Wr1", [F, HC], FP32, isOutput=False)
    We1_e = nc.declare_dram_parameter("We1", [EDIM, HC], FP32, isOutput=False)
    att1_e = nc.declare_dram_parameter("att1rep", [P, HC], FP32, isOutput=False)
    Wl2_e = nc.declare_dram_parameter("Wl2", [HC, H], FP32, isOutput=False)
    Wr2_e = nc.declare_dram_parameter("Wr2", [HC, H], FP32, isOutput=False)
    We2_e = nc.declare_dram_parameter("We2", [EDIM, H], FP32, isOutput=False)
    out_e = nc.declare_dram_parameter("out", [NLP, 1], FP32, isOutput=True)

    xl_loc = nc.dram_tensor("xl_loc", [NLP, HC], BF16)
    xr_loc = nc.dram_tensor("xr_loc", [NLP, HC], BF16)
    xl_full = nc.dram_tensor("xl_full", [NTAB, HC], BF16, addr_space="Shared")
    n2_loc = nc.dram_tensor("n2_loc", [NLP, P], BF16)
    n2_full = nc.dram_tensor("n2_full", [NTAB, P], BF16, addr_space="Shared")
    aggea_d = nc.dram_tensor("aggea_d", [NWIN, EDIM, WN], BF16)

    cc_sem = nc.alloc_semaphore("cc_sem")
    NT_LOC = NLP // P

    # =================== TC0: node features ===================
    with tile.TileContext(nc) as tc:
        with tc.tile_pool(name="c0", bufs=1) as cpool, \
             tc.tile_pool(name="s0", bufs=3) as spool, \
             tc.tile_pool(name="p0", bufs=3, space="PSUM") as ppool:
            wl1 = cpool.tile([F, HC], BF16)
            wr1 = cpool.tile([F, HC], BF16)
            nc.gpsimd.dma_start(wl1[:], Wl1_e[:, :])
            nc.gpsimd.dma_start(wr1[:], Wr1_e[:, :])
            for j in range(NT_LOC):
                xt = spool.tile([F, P], BF16, tag="xt")
                nc.gpsimd.dma_start(xt[:], xT_e[:, j * P:(j + 1) * P])
                ps = ppool.tile([P, 2, HC], FP32, tag="ps")
                nc.tensor.matmul(ps[:, 0], xt[:], wl1[:], start=True, stop=True)
                nc.tensor.matmul(ps[:, 1], xt[:], wr1[:], start=True, stop=True)
                sb = spool.tile([P, 2, HC], BF16, tag="sb")
                nc.vector.tensor_copy(sb[:], ps[:])
                nc.gpsimd.dma_start(xl_loc[j * P:(j + 1) * P, :], sb[:, 0])
                nc.gpsimd.dma_start(xr_loc[j * P:(j + 1) * P, :], sb[:, 1])

    nc.gpsimd.collective_compute(
        "AllGather", ALU.bypass, replica_groups=[list(range(NCORES))],
        ins=[xl_loc[:, :]], outs=[xl_full[:, :]],
    ).then_inc(cc_sem)
    nc.gpsimd.wait_ge(cc_sem, 1)

    # =================== TC1: layer-1 edge pass ===================
    with tile.TileContext(nc) as tc:
        with tc.tile_pool(name="c1", bufs=1) as cpool, \
             tc.tile_pool(name="g1", bufs=2) as gpool, \
             tc.tile_pool(name="s1", bufs=4) as spool, \
             tc.tile_pool(name="w1", bufs=2) as wpool, \
             tc.tile_pool(name="pe1", bufs=2, space="PSUM") as pepool, \
             tc.tile_pool(name="pg1", bufs=2, space="PSUM") as pgpool, \
             tc.tile_pool(name="px1", bufs=1, space="PSUM") as pxpool, \
             tc.tile_pool(name="pp1", bufs=1, space="PSUM") as pppool:
            ident = cpool.tile([P, P], BF16)
            make_identity(nc, ident[:])
            we1 = cpool.tile([EDIM, HC], BF16)
            nc.gpsimd.dma_start(we1[:], We1_e[:, :])
            att1rb = cpool.tile([P, HC], BF16)
            att1r = cpool.tile([P, HC], FP32)
            nc.gpsimd.dma_start(att1r[:], att1_e[:, :])
            nc.vector.tensor_copy(att1rb[:], att1r[:])
            iotai = cpool.tile([P, WN], I32)
            nc.gpsimd.iota(iotai[:], pattern=[[1, WN]], base=0,
                           channel_multiplier=0)
            iota = cpool.tile([P, WN], BF16)
            nc.vector.tensor_copy(iota[:], iotai[:])
            wl2c = cpool.tile([HC, 3, H], BF16)
            wtmp = cpool.tile([HC, 3, H], FP32)
            for k, wsrc in ((0, Wl2_e), (1, Wl2_e), (2, Wr2_e)):
                nc.gpsimd.dma_start(wtmp[:, k], wsrc[:, :])
            for hh in range(H):
                srch = int(h_ord[hh])
                nc.scalar.mul(wl2c[:, 0, hh:hh + 1], wtmp[:, 0, srch:srch + 1], 1.0)
                nc.scalar.mul(wl2c[:, 1, hh:hh + 1], wtmp[:, 1, srch:srch + 1],
                              float(s2[hh] * sgn2[hh]))
                nc.scalar.mul(wl2c[:, 2, hh:hh + 1], wtmp[:, 2, srch:srch + 1],
                              float(s2[hh] * sgn2[hh]))

            for w in range(NWIN):
                xlg = gpool.tile([P, T, HC], BF16, tag="xlg")
                giA = gpool.tile([P, TA * 8], I16, tag="giA")
                nc.gpsimd.dma_start(giA[:], gidxA_e[w])
                giB = gpool.tile([P, TB * 8], I16, tag="giB")
                nc.gpsimd.dma_start(giB[:], gidxB_e[w])
                nc.gpsimd.dma_gather(
                    out_ap=xlg[:, :TA], in_ap=xl_full[:HALF, :],
                    idxs_ap=giA[:], num_idxs=TA * P, num_idxs_reg=TA * P,
                    elem_size=HC)
                nc.gpsimd.dma_gather(
                    out_ap=xlg[:, TA:], in_ap=xl_full[HALF:, :],
                    idxs_ap=giB[:], num_idxs=TB * P, num_idxs_reg=TB * P,
                    elem_size=HC)
                rhsw = wpool.tile([P, HC], BF16, tag="rhsw")
                nc.vector.tensor_copy(rhsw[:EDIM], we1[:])
                nc.gpsimd.dma_start(rhsw[EDIM:EDIM + WN],
                                    xr_loc[w * WN:(w + 1) * WN, :])
                xlw = wpool.tile([WN, HC], BF16, tag="xlw")
                nc.gpsimd.dma_start(xlw[:], xl_loc[w * WN:(w + 1) * WN, :])
                xrw = wpool.tile([WN, HC], BF16, tag="xrw")
                nc.gpsimd.dma_start(xrw[:], xr_loc[w * WN:(w + 1) * WN, :])
                ivdw = wpool.tile([WN, 1], FP32, tag="ivdw")
                nc.gpsimd.dma_start(ivdw[:], ivd_e[w * WN:(w + 1) * WN])

                pagg = pgpool.tile([WN, PAY], FP32, tag="pagg")
                pea = pxpool.tile([EDIM, WN], FP32, tag="pea")

                for t in range(T):
                    statt = spool.tile([P, P], BF16, tag="stat")
                    nc.gpsimd.dma_start(statt[:], stat_e[w, t])
                    segt = spool.tile([P, 1], FP32, tag="seg")
                    nc.gpsimd.dma_start(segt[:], seg_e[w, t])
                    payload = spool.tile([P, PAY], BF16, tag="pay")
                    nc.gpsimd.dma_start(payload[:, HC + H:], ea_e[w, t])

                    pe = pepool.tile([P, HC], FP32, tag="pe")
                    nc.tensor.matmul(pe[:], statt[:], rhsw[:], start=True,
                                     stop=False)
                    nc.tensor.matmul(pe[:], ident[:], xlg[:, t], start=False,
                                     stop=True)

                    onehot = spool.tile([P, WN], BF16, tag="oh")
                    nc.vector.tensor_scalar(onehot[:], iota[:], segt[:], None,
                                            op0=ALU.is_equal)

                    el = spool.tile([P, HC], BF16, tag="el")
                    nc.scalar.activation(el[:], pe[:], ACTF.Prelu, alpha=NEG_ATT)
                    prod = spool.tile([P, HC], BF16, tag="prod")
                    nc.vector.tensor_mul(prod[:], el[:], att1rb[:])
                    logit = spool.tile([P, H], FP32, tag="logit")
                    nc.vector.tensor_reduce(
                        logit[:], prod[:].rearrange("p (h c) -> p h c", h=H),
                        axis=AX.X, op=ALU.add)
                    nc.scalar.activation(payload[:, HC:HC + H], logit[:], ACTF.Exp)
                    nc.vector.tensor_tensor(
                        out=payload[:, :HC].rearrange("p (h c) -> p h c", h=H),
                        in0=xlg[:, t].rearrange("p (h c) -> p h c", h=H),
                        in1=payload[:, HC:HC + H].to_broadcast([P, H, C]),
                        op=ALU.mult)

                    nc.tensor.matmul(pagg[:], onehot[:], payload[:],
                                     start=(t == 0), stop=(t == T - 1))
                    nc.tensor.matmul(pea[:], payload[:, HC + H:], onehot[:],
                                     start=(t == 0), stop=(t == T - 1))

                # ---- window epilogue ----
                aggs = wpool.tile([WN, PAY], FP32, tag="aggs")
                nc.vector.tensor_copy(aggs[:], pagg[:])
                aget = wpool.tile([EDIM, WN], BF16, tag="aget")
                nc.vector.tensor_copy(aget[:], pea[:])
                nc.gpsimd.dma_start(aggea_d[w], aget[:])

                ploop = pppool.tile([WN, HC], FP32, tag="epi")
                nc.tensor.matmul(ploop[:], aget[:], we1[:], start=True, stop=True)
                loop_sc = wpool.tile([WN, HC], BF16, tag="loopsc")
                nc.scalar.activation(loop_sc[:], ploop[:], ACTF.Copy,
                                     scale=ivdw[:])

                pself = pppool.tile([WN, HC], FP32, tag="epi")
                nc.tensor.matmul(pself[:], ident[:WN, :WN], xlw[:],
                                 start=True, stop=False)
                nc.tensor.matmul(pself[:], ident[:WN, :WN], xrw[:],
                                 start=False, stop=False)
                nc.tensor.matmul(pself[:], ident[:WN, :WN], loop_sc[:],
                                 start=False, stop=True)
                esl = wpool.tile([WN, HC], BF16, tag="esl")
                nc.scalar.activation(esl[:], pself[:], ACTF.Prelu, alpha=NEG_ATT)
                prods = wpool.tile([WN, HC], BF16, tag="prods")
                nc.vector.tensor_mul(prods[:], esl[:], att1rb[:WN])
                lgs = wpool.tile([WN, H], FP32, tag="lgs")
                nc.vector.tensor_reduce(
                    lgs[:], prods[:].rearrange("p (h c) -> p h c", h=H),
                    axis=AX.X, op=ALU.add)
                exs = wpool.tile([WN, H], FP32, tag="exs")
                nc.scalar.activation(exs[:], lgs[:], ACTF.Exp)

                den = wpool.tile([WN, H], FP32, tag="den")
                nc.vector.tensor_add(den[:], aggs[:, HC:HC + H], exs[:])
                nc.vector.reciprocal(den[:], den[:])
                selfw = wpool.tile([WN, HC], FP32, tag="selfw")
                nc.vector.tensor_tensor(
                    out=selfw[:].rearrange("p (h c) -> p h c", h=H),
                    in0=xlw[:].rearrange("p (h c) -> p h c", h=H),
                    in1=exs[:].to_broadcast([WN, H, C]),
                    op=ALU.mult)
                nc.vector.tensor_add(selfw[:], selfw[:], aggs[:, :HC])
                hw_t = wpool.tile([WN, HC], FP32, tag="hw")
                nc.vector.tensor_tensor(
                    out=hw_t[:].rearrange("p (h c) -> p h c", h=H),
                    in0=selfw[:].rearrange("p (h c) -> p h c", h=H),
                    in1=den[:].to_broadcast([WN, H, C]),
                    op=ALU.mult)
                hwb = wpool.tile([WN, HC], BF16, tag="hwb")
                nc.scalar.activation(hwb[:], hw_t[:], ACTF.Prelu, alpha=NEG_ACT)

                phT = pppool.tile([HC, WN], BF16, tag="epiT")
                nc.tensor.transpose(phT[:], hwb[:], ident[:WN, :WN])
                hT = wpool.tile([HC, WN], BF16, tag="hT")
                nc.vector.tensor_copy(hT[:], phT[:])
                pn2 = pppool.tile([WN, 3 * H], FP32, tag="epi")
                nc.tensor.matmul(pn2[:], hT[:], wl2c[:], start=True, stop=True)
                n2 = wpool.tile([WN, P], BF16, tag="n2")
                nc.gpsimd.memset(n2[:], 0.0)
                nc.vector.tensor_copy(n2[:, :3 * H], pn2[:])
                nc.gpsimd.dma_start(n2_loc[w * WN:(w + 1) * WN, :], n2[:])

    nc.gpsimd.collective_compute(
        "AllGather", ALU.bypass, replica_groups=[list(range(NCORES))],
        ins=[n2_loc[:, :]], outs=[n2_full[:, :]],
    ).then_inc(cc_sem)
    nc.gpsimd.wait_ge(cc_sem, 2)

    # =================== TC2: layer-2 edge pass ===================
    with tile.TileContext(nc) as tc:
        with tc.tile_pool(name="c2", bufs=1) as cpool, \
             tc.tile_pool(name="g2", bufs=2) as gpool, \
             tc.tile_pool(name="s2", bufs=4) as spool, \
             tc.tile_pool(name="w2", bufs=2) as wpool, \
             tc.tile_pool(name="pz2", bufs=2, space="PSUM") as pzpool, \
             tc.tile_pool(name="pg2", bufs=2, space="PSUM") as pgpool, \
             tc.tile_pool(name="pp2", bufs=2, space="PSUM") as pppool:
            ident = cpool.tile([P, P], BF16)
            make_identity(nc, ident[:])
            iotai = cpool.tile([P, WN], I32)
            nc.gpsimd.iota(iotai[:], pattern=[[1, WN]], base=0,
                           channel_multiplier=0)
            iota = cpool.tile([P, WN], BF16)
            nc.vector.tensor_copy(iota[:], iotai[:])
            we2a = cpool.tile([EDIM, H], BF16)
            wtmp2 = cpool.tile([EDIM, H], FP32)
            nc.gpsimd.dma_start(wtmp2[:], We2_e[:, :])
            for hh in range(H):
                srch = int(h_ord[hh])
                nc.scalar.mul(we2a[:, hh:hh + 1], wtmp2[:, srch:srch + 1],
                              float(s2[hh] * sgn2[hh]))

            for w in range(NWIN):
                xg2 = gpool.tile([P, T, P], BF16, tag="xg2")
                giA = gpool.tile([P, TA * 8], I16, tag="giA")
                nc.gpsimd.dma_start(giA[:], gidxA_e[w])
                giB = gpool.tile([P, TB * 8], I16, tag="giB")
                nc.gpsimd.dma_start(giB[:], gidxB_e[w])
                nc.gpsimd.dma_gather(
                    out_ap=xg2[:, :TA], in_ap=n2_full[:HALF, :],
                    idxs_ap=giA[:], num_idxs=TA * P, num_idxs_reg=TA * P,
                    elem_size=P)
                nc.gpsimd.dma_gather(
                    out_ap=xg2[:, TA:], in_ap=n2_full[HALF:, :],
                    idxs_ap=giB[:], num_idxs=TB * P, num_idxs_reg=TB * P,
                    elem_size=P)
                n2w = wpool.tile([WN, 16], BF16, tag="n2w")
                nc.gpsimd.dma_start(n2w[:], n2_loc[w * WN:(w + 1) * WN, :16])
                aget = wpool.tile([EDIM, WN], BF16, tag="aget")
                nc.gpsimd.dma_start(aget[:], aggea_d[w])
                ivdw = wpool.tile([WN, 1], FP32, tag="ivdw")
                nc.gpsimd.dma_start(ivdw[:], ivd_e[w * WN:(w + 1) * WN])
                # combined rhs for the z matmul: [We2att(16); xr2att_win(112)]
                rhs2 = wpool.tile([P, H], BF16, tag="rhs2")
                nc.vector.tensor_copy(rhs2[:EDIM], we2a[:])
                nc.gpsimd.dma_start(rhs2[EDIM:EDIM + WN], n2w[:, 8:12])

                pagg = pgpool.tile([WN, 2 * H], FP32, tag="pagg")

                for t in range(T):
                    segt = spool.tile([P, 1], FP32, tag="seg")
                    nc.gpsimd.dma_start(segt[:], seg_e[w, t])
                    statt = spool.tile([P, P], BF16, tag="stat")
                    nc.gpsimd.dma_start(statt[:], stat_e[w, t])
                    onehot = spool.tile([P, WN], BF16, tag="oh")
                    nc.vector.tensor_scalar(onehot[:], iota[:], segt[:], None,
                                            op0=ALU.is_equal)

                    pz = pzpool.tile([P, H], FP32, tag="pz")
                    nc.tensor.matmul(pz[:], statt[:], rhs2[:], start=True,
                                     stop=False)
                    nc.tensor.matmul(pz[:], ident[:], xg2[:, t, 4:8],
                                     start=False, stop=True)

                    pay2 = spool.tile([P, 2 * H], BF16, tag="pay2")
                    zl = spool.tile([P, H], FP32, tag="zl")
                    if npos2 > 0:
                        nc.scalar.activation(zl[:, :npos2], pz[:, :npos2],
                                             ACTF.Prelu, alpha=NEG_ATT)
                    if npos2 < H:
                        nc.scalar.activation(zl[:, npos2:], pz[:, npos2:],
                                             ACTF.Prelu, alpha=1.0 / NEG_ATT)
                    nc.scalar.activation(pay2[:, H:], zl[:], ACTF.Exp)
                    nc.vector.tensor_tensor(out=pay2[:, :H], in0=xg2[:, t, 0:4],
                                            in1=pay2[:, H:], op=ALU.mult)
                    nc.tensor.matmul(pagg[:], onehot[:], pay2[:],
                                     start=(t == 0), stop=(t == T - 1))

                # ---- window epilogue ----
                aggs = wpool.tile([WN, 2 * H], FP32, tag="aggs")
                nc.vector.tensor_copy(aggs[:], pagg[:])
                ploop = pppool.tile([WN, H], FP32, tag="epi")
                nc.tensor.matmul(ploop[:], aget[:], we2a[:], start=True,
                                 stop=True)
                loop_sc = wpool.tile([WN, H], BF16, tag="loopsc")
                nc.scalar.activation(loop_sc[:], ploop[:], ACTF.Copy,
                                     scale=ivdw[:])
                pself = pppool.tile([WN, H], FP32, tag="epi")
                nc.tensor.matmul(pself[:], ident[:WN, :WN], n2w[:, 4:8],
                                 start=True, stop=False)
                nc.tensor.matmul(pself[:], ident[:WN, :WN], n2w[:, 8:12],
                                 start=False, stop=False)
                nc.tensor.matmul(pself[:], ident[:WN, :WN], loop_sc[:],
                                 start=False, stop=True)
                zls = wpool.tile([WN, H], FP32, tag="zls")
                if npos2 > 0:
                    nc.scalar.activation(zls[:, :npos2], pself[:, :npos2],
                                         ACTF.Prelu, alpha=NEG_ATT)
                if npos2 < H:
                    nc.scalar.activation(zls[:, npos2:], pself[:, npos2:],
                                         ACTF.Prelu, alpha=1.0 / NEG_ATT)
                exs = wpool.tile([WN, H], FP32, tag="exs")
                nc.scalar.activation(exs[:], zls[:], ACTF.Exp)
                den = wpool.tile([WN, H], FP32, tag="den")
                nc.vector.tensor_add(den[:], aggs[:, H:], exs[:])
                nc.vector.reciprocal(den[:], den[:])
                sw = wpool.tile([WN, H], FP32, tag="sw")
                nc.vector.tensor_mul(sw[:], n2w[:, 0:4], exs[:])
                nc.vector.tensor_add(sw[:], sw[:], aggs[:, :H])
                nc.vector.tensor_mul(sw[:], sw[:], den[:])
                oo = wpool.tile([WN, 1], FP32, tag="oo")
                nc.vector.tensor_reduce(oo[:], sw[:], axis=AX.X, op=ALU.add)
                oo2 = wpool.tile([WN, 1], FP32, tag="oo2")
                nc.scalar.mul(oo2[:], oo[:], 1.0 / H)
                nc.gpsimd.dma_start(out_e[w * WN:(w + 1) * WN], oo2[:])

    return nc


def kernel(x, edge_index, edge_attr, Wl1, Wr1, We1, att1, Wl2, Wr2, We2, att2,
           _want_exec_time=[None]):
    x = np.asarray(x); edge_index = np.asarray(edge_index)
    edge_attr = np.asarray(edge_attr)
    att1 = np.asarray(att1); att2 = np.asarray(att2)
    maps, meta = _prep(x, edge_index, edge_attr)

    from concourse.bass_utils import run_bass_kernel_spmd
    nc = _build(meta, att2)

    att1rep = np.tile(np.asarray(att1).reshape(1, -1), (P, 1)).astype(np.float32)
    in_maps = []
    for c in range(NCORES):
        m = maps[c]
        in_maps.append({
            "xT": m["xT"], "gidxA": m["gidxA"], "gidxB": m["gidxB"],
            "seg": m["seg"], "stat": m["stat"],
            "ea": m["ea"], "invdeg": m["invdeg"],
            "Wl1": np.asarray(Wl1, np.float32), "Wr1": np.asarray(Wr1, np.float32),
            "We1": np.asarray(We1, np.float32), "att1rep": att1rep,
            "Wl2": np.asarray(Wl2, np.float32), "Wr2": np.asarray(Wr2, np.float32),
            "We2": np.asarray(We2, np.float32),
        })

    nc.compile()
    import os, time
    trace = os.environ.get('KTRACE', '0') == '1'
    t0 = time.perf_counter()
    res = run_bass_kernel_spmd(nc, in_maps, list(range(NCORES)),
                               trace=trace)
    t1 = time.perf_counter()
    et = getattr(res, "exec_time_ns", None)
    # NTFF profiling is unavailable in this container; fall back to the
    # wall-clock of the execute call (includes dispatch + input staging).
    _want_exec_time[0] = et if et else int((t1 - t0) * 1e9)
    NL = meta["NL"]
    outs = [np.asarray(res.results[c]["out"])[:NL, 0] for c in range(NCORES)]
    return np.concatenate(outs).astype(np.float32)


if __name__ == "__main__":
    import reference
    inputs = {k: np.asarray(v) for k, v in reference.setup_inputs().items()}
    got = kernel(**inputs)
    exp = np.asarray(reference.reference(**inputs))
    rel = np.linalg.norm(got - exp) / max(1e-12, np.linalg.norm(exp))
    print("rel-l2:", rel)

